# revision 1
# baseline (speedup 1.0000x reference)
"""ConvAttention Trainium2 kernel.

Data-parallel over batch: 8 batch elements -> 8 NeuronCores, no collectives.
Per core: depthwise3x3+BN+pointwise projections (BN folded on host), 8-head
attention, output linear. All matmuls run as float32r (full-rate fp32).
"""

import copy

import numpy as np

import concourse.bass as bass
import concourse.mybir as mybir
import concourse.tile as tile
from concourse.bass_utils import run_bass_kernel_spmd
from concourse.masks import make_identity

F32 = mybir.dt.float32
F32R = mybir.dt.float32r
AF = mybir.ActivationFunctionType

N_CORES = 8
DIM = 512      # channels
IMG = 32       # image side
NPIX = 1024    # pixels
HEADS = 8
DH = 64
PAD = 34       # padded image side
PADN = PAD * PAD
CB = 4         # channel blocks of 128
NB = 8         # pixel blocks of 128
NH = 2         # pixel halves of 512
BN_EPS = 1e-5


def _r(ap):
    return ap.bitcast(F32R)


def split_excess_waits(nc, lim=1):
    """walrus here rejects >1 sync wait per instruction; move extras onto
    NOPs inserted just before, on the same engine."""
    counter = 0
    new_module = copy.replace(nc.m, functions=[])
    for function in nc.m.functions:
        new_function = copy.replace(function, blocks=[])
        new_function.set_allocations_from_list(function.allocations)
        for block in function.blocks:
            insts = []
            for inst in block.instructions:
                si = inst.sync_info
                if si is not None and len(si.on_wait) > lim:
                    waits = list(si.on_wait)
                    extra, keep = waits[:-lim], waits[-lim:]
                    for w in extra:
                        counter += 1
                        nop = mybir.InstNoOp(
                            name=f"I-ws-{counter}", engine=inst.engine
                        )
                        nop.sync_info = mybir.SyncInfo(on_wait=[w], on_update=[])
                        insts.append(nop)
                    inst.sync_info = mybir.SyncInfo(
                        on_wait=keep, on_update=si.on_update
                    )
                insts.append(inst)
            new_function.blocks.append(copy.replace(block, instructions=insts))
        new_module.functions.append(new_function)
    nc.m = new_module
    return counter


def build_nc(waitfix=True):
    nc = bass.Bass(trn_type="TRN2")
    x_d = nc.declare_dram_parameter("x", [NPIX, DIM], F32, isOutput=False)
    pwt_d = nc.declare_dram_parameter("pwt", [3, DIM, DIM], F32, isOutput=False)
    pbqk_d = nc.declare_dram_parameter("pbqk", [2, DIM], F32, isOutput=False)
    dww_d = nc.declare_dram_parameter("dww", [3, DIM, 9], F32, isOutput=False)
    owt_d = nc.declare_dram_parameter("owt", [DIM, DIM], F32, isOutput=False)
    ob_d = nc.declare_dram_parameter("obias", [DIM], F32, isOutput=False)
    out_d = nc.declare_dram_parameter("out", [NPIX, DIM], F32, isOutput=True)

    with tile.TileContext(nc) as tc:
        with (
            tc.tile_pool(name="consts", bufs=1) as consts,
            tc.tile_pool(name="persist", bufs=1) as persist,
        ):
            # ---- constants ----
            ident = consts.tile([128, 128], F32, tag="ident", name="ident")
            make_identity(nc, ident[:, :])
            ostage = consts.tile([128, 64], F32, tag="ostage", name="ostage")
            nc.vector.memset(ostage[:, :], 1.0)
            ones_t = consts.tile([128, 64], F32R, tag="ones", name="ones")
            nc.gpsimd.tensor_copy(ones_t[:, :], ostage[:, :])
            ob_bc = consts.tile([128, DIM], F32, tag="ob_bc", name="ob_bc")
            ob_ap = ob_d[:]
            nc.sync.dma_start(
                out=ob_bc[:, :],
                in_=bass.AP(
                    tensor=ob_ap.tensor, offset=ob_ap.offset,
                    ap=[[0, 128]] + list(ob_ap.ap),
                ),
            )
            pb_t = []  # [i][ob] -> [128,1] bias tiles for q,k projections
            for i in range(2):
                row = []
                for ob in range(4):
                    t = consts.tile([128, 1], F32, tag=f"pb{i}_{ob}", name=f"pb{i}_{ob}")
                    nc.sync.dma_start(
                        out=t[:, :],
                        in_=pbqk_d[i, ob * 128:(ob + 1) * 128].rearrange(
                            "(p o) -> p o", o=1
                        ),
                    )
                    row.append(t)
                pb_t.append(row)
            dww_t = []  # [i*4+cb] -> [128,9]
            for i in range(3):
                for cb in range(CB):
                    t = consts.tile([128, 9], F32, tag=f"dw{i}_{cb}", name=f"dw{i}_{cb}")
                    nc.sync.dma_start(
                        out=t[:, :], in_=dww_d[i, cb * 128:(cb + 1) * 128, :]
                    )
                    dww_t.append(t)
            owt_sb = []
            for ob in range(4):
                ts = consts.tile([128, DIM], F32, tag=f"owts{ob}", name=f"owts{ob}")
                nc.sync.dma_start(
                    out=ts[:, :], in_=owt_d[ob * 128:(ob + 1) * 128, :]
                )
                t = consts.tile([128, DIM], F32R, tag=f"owt{ob}", name=f"owt{ob}")
                nc.gpsimd.tensor_copy(t[:, :], ts[:, :])
                owt_sb.append(t)

            # ---- persistent activations ----
            zstage = consts.tile([128, PADN], F32, tag="zstage", name="zstage")
            nc.gpsimd.memset(zstage[:, :], 0.0)
            xp = [persist.tile([128, PADN], F32R, tag=f"xp{cb}", name=f"xp{cb}") for cb in range(CB)]
            xp3 = [t[:, :].rearrange("p (r s) -> p r s", s=PAD) for t in xp]
            for t in xp:
                nc.gpsimd.tensor_copy(t[:, :], zstage[:, :])
            qT = [persist.tile([128, NPIX], F32R, tag=f"qT{ob}", name=f"qT{ob}") for ob in range(4)]
            kT = [persist.tile([128, NPIX], F32R, tag=f"kT{ob}", name=f"kT{ob}") for ob in range(4)]
            v_sb = [persist.tile([128, HEADS * 65], F32R, tag=f"v{nb}", name=f"v{nb}") for nb in range(NB)]
            for t in v_sb:
                # ones column per head (augmented V -> softmax denominators)
                nc.gpsimd.tensor_copy(
                    t[:, :].rearrange("p (h s) -> p h s", s=65)[:, :, 64:65],
                    ostage[:, 0:8].rearrange("p (h o) -> p h o", o=1),
                )
            outT = [persist.tile([128, NPIX], F32R, tag=f"oT{ob}", name=f"oT{ob}") for ob in range(4)]

            # ---- phase A: load x, transpose into padded [c, 34x34] image ----
            with (
                tc.tile_pool(name="xn", bufs=4) as xnp,
                tc.tile_pool(name="psA", bufs=4, space="PSUM") as psA,
            ):
                for nb in range(NB):
                    xn = xnp.tile([128, DIM], F32, tag="xn", name="xn")
                    nc.sync.dma_start(
                        out=xn[:, :], in_=x_d[nb * 128:(nb + 1) * 128, :]
                    )
                    for cb in range(CB):
                        pst = psA.tile([128, 128], F32, tag="pst", name="pst")
                        nc.tensor.transpose(
                            pst[:, :], xn[:, cb * 128:(cb + 1) * 128], ident[:, :]
                        )
                        nc.vector.tensor_copy(
                            xp3[cb][:, 1 + nb * 4:1 + nb * 4 + 4, 1:33],
                            pst[:, :].rearrange("p (a b) -> p a b", b=32),
                        )

            # ---- phases B+C: depthwise conv + pointwise per projection ----
            with (
                tc.tile_pool(name="diag", bufs=20) as diagp,
                tc.tile_pool(name="pwt", bufs=2) as pwtp,
                tc.tile_pool(name="ysb", bufs=1) as yp,
                tc.tile_pool(name="psY", bufs=2, space="PSUM") as psY,
                tc.tile_pool(name="psP", bufs=2, space="PSUM") as psP,
            ):
                for i in range(3):
                    pwt_sb = []
                    for cb in range(CB):
                        ts = pwtp.tile([128, DIM], F32, tag=f"pwts{cb}", name=f"pwts{cb}")
                        nc.sync.dma_start(
                            out=ts[:, :], in_=pwt_d[i, cb * 128:(cb + 1) * 128, :]
                        )
                        t = pwtp.tile([128, DIM], F32R, tag=f"pwt{cb}", name=f"pwt{cb}")
                        nc.gpsimd.tensor_copy(t[:, :], ts[:, :])
                        pwt_sb.append(t)
                    y_sb = [yp.tile([128, NPIX], F32R, tag=f"y{cb}", name=f"y{cb}") for cb in range(CB)]
                    for cb in range(CB):
                        dwt = dww_t[i * 4 + cb]
                        diags = []
                        for t in range(9):
                            dg = diagp.tile([128, 128], F32R, tag="dg", name="dg")
                            nc.vector.tensor_scalar_mul(
                                dg[:, :], ident[:, :], dwt[:, t:t + 1]
                            )
                            diags.append(dg)
                        for nh in range(NH):
                            yps = psY.tile([128, 512], F32, tag="yps", name="yps")
                            for t in range(9):
                                kh, kw = t // 3, t % 3
                                rhs = xp3[cb][
                                    :, nh * 16 + kh:nh * 16 + kh + 16, kw:kw + 32
                                ]
                                nc.tensor.matmul(
                                    yps[:, :], _r(diags[t][:, :]), _r(rhs),
                                    start=(t == 0), stop=(t == 8),
                                )
                            nc.scalar.copy(
                                y_sb[cb][:, nh * 512:(nh + 1) * 512], yps[:, :]
                            )
                    if i < 2:
                        dst = qT if i == 0 else kT
                        for ob in range(4):
                            for nh in range(NH):
                                pp = psP.tile([128, 512], F32, tag="pp", name="pp")
                                for cb in range(CB):
                                    nc.tensor.matmul(
                                        pp[:, :],
                                        _r(pwt_sb[cb][:, ob * 128:(ob + 1) * 128]),
                                        _r(y_sb[cb][:, nh * 512:(nh + 1) * 512]),
                                        start=(cb == 0), stop=(cb == 3),
                                    )
                                nc.scalar.activation(
                                    dst[ob][:, nh * 512:(nh + 1) * 512], pp[:, :],
                                    AF.Identity, bias=pb_t[i][ob][:, :], scale=1.0,
                                )
                    else:
                        for nb in range(NB):
                            pv = psP.tile([128, 512], F32, tag="pp", name="pp")
                            for cb in range(CB):
                                nc.tensor.matmul(
                                    pv[:, :],
                                    _r(y_sb[cb][:, nb * 128:(nb + 1) * 128]),
                                    _r(pwt_sb[cb][:, :]),
                                    start=(cb == 0), stop=(cb == 3),
                                )
                            nc.scalar.copy(
                                v_sb[nb][:, :].rearrange(
                                    "p (h s) -> p h s", s=65
                                )[:, :, 0:64],
                                pv[:, :].rearrange("p (h s) -> p h s", s=64),
                            )

            # ---- phase D: attention per head ----
            with (
                tc.tile_pool(name="psS", bufs=2, space="PSUM") as psS,
                tc.tile_pool(name="expT", bufs=10) as expp,
                tc.tile_pool(name="psAV", bufs=2, space="PSUM") as psAV,
                tc.tile_pool(name="psBC", bufs=2, space="PSUM") as psBC,
                tc.tile_pool(name="recp", bufs=2) as recp,
                tc.tile_pool(name="recB", bufs=3) as recBp,
                tc.tile_pool(name="tmpp", bufs=2) as tmpp,
            ):
                for h in range(HEADS):
                    ob, loc = h // 2, (h % 2) * 64
                    expt = []
                    for mb in range(NB):
                        sp = psS.tile([128, NPIX], F32, tag="sp", name="sp")
                        for nh in range(NH):
                            nc.tensor.matmul(
                                sp[:, nh * 512:(nh + 1) * 512],
                                _r(kT[ob][loc:loc + 64, mb * 128:(mb + 1) * 128]),
                                _r(qT[ob][loc:loc + 64, nh * 512:(nh + 1) * 512]),
                                start=True, stop=True,
                            )
                        et = expp.tile([128, NPIX], F32R, tag="expT", name="expT")
                        nc.scalar.activation(
                            et[:, :], sp[:, :], AF.Exp, scale=float(DH) ** -0.5
                        )
                        expt.append(et)
                    avs = []
                    for nh in range(NH):
                        av = psAV.tile([65, 512], F32, tag="av", name="av")
                        for mb in range(NB):
                            nc.tensor.matmul(
                                av[:, :],
                                _r(v_sb[mb][:, h * 65:(h + 1) * 65]),
                                _r(expt[mb][:, nh * 512:(nh + 1) * 512]),
                                start=(mb == 0), stop=(mb == 7),
                            )
                        avs.append(av)
                    rec = recp.tile([65, NPIX], F32R, tag="rec", name="rec")
                    with nc.allow_low_precision(reason="fp32r softmax denom"):
                        for nh in range(NH):
                            nc.vector.reciprocal(
                                rec[64:65, nh * 512:(nh + 1) * 512], avs[nh][64:65, :]
                            )
                    dst = outT[ob] if loc == 0 else tmpp.tile([64, NPIX], F32R, tag="tmp", name="tmp")
                    for nh in range(NH):
                        bc = psBC.tile([64, 512], F32, tag="bc", name="bc")
                        nc.tensor.matmul(
                            bc[:, :], _r(ones_t[64:65, :]),
                            _r(rec[64:65, nh * 512:(nh + 1) * 512]),
                            start=True, stop=True,
                        )
                        rB = recBp.tile([64, 512], F32, tag="recB", name="recB")
                        nc.vector.tensor_copy(rB[:, :], bc[:, :])
                        nc.vector.tensor_mul(
                            dst[0:64, nh * 512:(nh + 1) * 512],
                            avs[nh][0:64, :], rB[:, :],
                        )
                    if loc:
                        nc.sync.dma_start(out=outT[ob][64:128, :], in_=dst[0:64, :])

            # ---- phase E: output linear + bias, store ----
            with (
                tc.tile_pool(name="psZ", bufs=2, space="PSUM") as psZ,
                tc.tile_pool(name="zsb", bufs=3) as zp,
            ):
                for nb in range(NB):
                    zps = psZ.tile([128, 512], F32, tag="zps", name="zps")
                    for ob in range(4):
                        nc.tensor.matmul(
                            zps[:, :],
                            _r(outT[ob][:, nb * 128:(nb + 1) * 128]),
                            _r(owt_sb[ob][:, :]),
                            start=(ob == 0), stop=(ob == 3),
                        )
                    zt = zp.tile([128, 512], F32, tag="zt", name="zt")
                    nc.vector.tensor_add(zt[:, :], zps[:, :], ob_bc[:, :])
                    nc.sync.dma_start(
                        out=out_d[nb * 128:(nb + 1) * 128, :], in_=zt[:, :]
                    )

    if waitfix:
        split_excess_waits(nc)
    return nc


def prep_weights(dw_w, dw_b, bn_g, bn_b, bn_m, bn_v, pw_w, pw_b, out_w, out_b):
    dw_w = np.asarray(dw_w, np.float32)
    s = np.asarray(bn_g, np.float32) / np.sqrt(np.asarray(bn_v, np.float32) + BN_EPS)
    pw_w = np.asarray(pw_w, np.float32)
    PW = pw_w * s[:, None, :]                       # [3, o, c]
    beta = (np.asarray(dw_b, np.float32) - np.asarray(bn_m, np.float32)) * s \
        + np.asarray(bn_b, np.float32)              # [3, c]
    pb = np.einsum("ioc,ic->io", pw_w, beta) + np.asarray(pw_b, np.float32)
    out_w = np.asarray(out_w, np.float32)
    return {
        "pwt": np.ascontiguousarray(PW.transpose(0, 2, 1)),      # [3, c, o]
        "pbqk": np.ascontiguousarray(pb[:2]),
        "dww": np.ascontiguousarray(dw_w.reshape(3, DIM, 9)),
        "owt": np.ascontiguousarray(out_w.T),                    # [o, c_out]
        "obias": (np.asarray(out_b, np.float32) + out_w @ pb[2]).astype(np.float32),
    }


_NC_CACHE = []


def kernel(x, dw_w, dw_b, bn_g, bn_b, bn_m, bn_v, pw_w, pw_b, out_w, out_b):
    x = np.asarray(x, np.float32)
    w = prep_weights(dw_w, dw_b, bn_g, bn_b, bn_m, bn_v, pw_w, pw_b, out_w, out_b)
    if not _NC_CACHE:
        _NC_CACHE.append(build_nc())
    nc = _NC_CACHE[0]
    in_maps = [dict(w, x=np.ascontiguousarray(x[b])) for b in range(N_CORES)]
    res = run_bass_kernel_spmd(nc, in_maps, list(range(N_CORES)))
    return np.stack([res.results[b]["out"] for b in range(N_CORES)], axis=0)

